# revision 1
# baseline (speedup 1.0000x reference)
"""CFConv (SchNet continuous-filter convolution) Trainium2 Bass kernel.

Problem: nn_CFConv_44332652429581 (gnn_message_passing, 8 cores).

Reference computation (per batch element b):
    W    = ssp(f_ij @ W1 + b1) @ W2 + b2          # filter net, (A,NBH,F)
    C    = 0.5*(cos(pi*r/5)+1)*(r<5)              # cosine cutoff, (A,NBH)
    Wc   = W * C * mask
    y    = x @ W_in2f                              # (A,F)
    agg  = sum_n  y[nbh[a,n]] * Wc[a,n]            # (A,F)
    out  = ssp(agg @ W_out + b_out)                # (A,O)
where ssp(v) = softplus(v) - ln 2.

Sharding: data-parallel over the batch axis, one batch element per core
(B=8 == n_cores). No collectives.

Per-core dataflow (pairs = A*NBH = 32768; "chunk" = one atom's 128
neighbors; "block" = 4096 pairs = 32 atoms). Blocks are pipelined:
load f_ij block / mm1 / ssp, gather that block's neighbor rows, then
filter-multiply-reduce it, all stages overlapping across blocks.

  - mm1 F-major: psum_h = W1^T @ f_ij^T            (bf16, W1 stationary)
  - ssp exactly in two ACT passes: u = Exp(h1+b1); h' = Ln(0.5u+0.5)
    (Ln(0.5e^v+0.5) == softplus(v) - ln2 exactly, so W = h'@W2 + b2)
  - y table (A,F) bf16 in DRAM; neighbor rows fetched with dma_gather,
    1024 rows per instruction (SWDGE ring holds 128 descs/engine)
    -> yg pairs-major, chunk == atom
  - mm2 per chunk: lhsT = h' slice (weights), rhs = W2 -> M2 (pairs,F) PSUM
  - V = yg * M2 on DVE (one pass per 512-pair superchunk)
  - neighbor reduce as one matmul per atom: aggT[:, a] = V_a^T @ Cm_a
    (the cutoff+mask vector Cm rides along as the reduce weights)
  - b2 correction (only when b2 != 0): agg += b2 (x)_f R, R^T = y^T @ T^T
    with T[a,j] = sum_{n: nbh[a,n]=j} Cm[a,n] host-precomputed from
    (neighbors, r_ij, mask) -- pure input preprocessing.
  - out-proj: psum_o = W_out^T @ aggT, final ssp via Exp/Ln,
    store outT (O, A); host transposes per core on unshard.

All DMA goes through gpsimd (SWDGE): the HWDGE rings (sync/scalar
engines) do not function on the axon PJRT runtime this kernel targets.
"""
import math
import os

import numpy as np
import ml_dtypes

import concourse.bass as bass
import concourse.tile as tile
from concourse import bacc, mybir
from concourse.bass_utils import run_bass_kernel_spmd


def _patch_act_tables():
    """Prefer the combined Exp+Ln activation table so the ACT engine does
    not thrash 1.3us table reloads between the softplus Exp and Ln passes."""
    if getattr(bacc, "_cfconv_act_patch", False):
        return
    orig = bacc.get_activation_tables

    def patched(arch):
        # Table ids are positional: keep the dict order identical, but strip
        # Exp/Ln from the single-transcendental tables so the chooser must
        # pick the combined natural_log_exp table for both passes.
        t = dict(orig(arch))
        exp_t = mybir.ActivationFunctionType.Exp
        ln_t = mybir.ActivationFunctionType.Ln
        out = {}
        for k, funcs in t.items():
            if k != "natural_log_exp_and_others" and (
                    exp_t in funcs or ln_t in funcs):
                funcs = funcs - {exp_t, ln_t}
            out[k] = funcs
        return out

    bacc.get_activation_tables = patched
    bacc._cfconv_act_patch = True

F32 = mybir.dt.float32
BF16 = mybir.dt.bfloat16
I16 = mybir.dt.int16

B, A, NBH = 8, 256, 128
G, F, O = 25, 128, 128
PAIRS = A * NBH            # 32768
CUTOFF = 5.0
NBLK = 8                   # pipeline blocks of 4096 pairs
NSC_BLK = 8                # superchunks (512 pairs) per block

# bf16 const-pack column layout
BC_XT = 0          # xT (128, 256)
BC_W2 = 256        # W2 (128, 128)
BC_WIN = 384       # W_in2f (128, 128)
BC_WOUT = 512      # W_out (128, 128)
BC_W1 = 640        # W1 padded to (128, 128); rows 0:25 valid
BC_TT = 768        # TT (2x (128, 256)) when use_b2

FC_RT = 0          # rT (128, 256) in the f32 pack

_prog_cache = {}
_runner_cache = {}
_last_results = None       # test.py introspection


def _build(use_b2: bool, use_mask: bool):
    """Build + compile the per-core Bass program (SPMD; same for all cores)."""
    from contextlib import ExitStack

    nbf = 768 + (512 if use_b2 else 0)
    fc_mask = 256
    fc_sc = 256 + (256 if use_mask else 0)   # scalar column block start
    nf32 = fc_sc + 5
    C_B1, C_BOUT, C_PIH, C_HALF, C_B2 = range(fc_sc, fc_sc + 5)

    _patch_act_tables()
    nc = bacc.Bacc("TRN2")

    fijT_d = nc.dram_tensor("fijT", [G, PAIRS], BF16, kind="ExternalInput")
    idx_d = nc.dram_tensor("idx16", [128, PAIRS // 16], I16, kind="ExternalInput")
    bfp_d = nc.dram_tensor("bfpack", [128, nbf], BF16, kind="ExternalInput")
    fp_d = nc.dram_tensor("fpack", [128, nf32], F32, kind="ExternalInput")

    outT_d = nc.dram_tensor("outT", [O, A], F32, kind="ExternalOutput")
    ytab_d = nc.dram_tensor("ytab", [A, F], BF16)  # internal

    with tile.TileContext(nc) as tc:
        with ExitStack() as ctx:
            const = ctx.enter_context(tc.tile_pool(name="const", bufs=1))
            fpool = ctx.enter_context(tc.tile_pool(name="fij", bufs=4))
            upool = ctx.enter_context(tc.tile_pool(name="u", bufs=3))
            hpool = ctx.enter_context(tc.tile_pool(name="hh", bufs=1))
            ygpool = ctx.enter_context(tc.tile_pool(name="yg", bufs=1))
            vpool = ctx.enter_context(tc.tile_pool(name="v", bufs=6))
            mpool = ctx.enter_context(tc.tile_pool(name="misc", bufs=1))
            ps_h = ctx.enter_context(tc.tile_pool(name="psh", bufs=2, space="PSUM"))
            ps_m2 = ctx.enter_context(tc.tile_pool(name="psm2", bufs=2, space="PSUM"))
            ps_agg = ctx.enter_context(tc.tile_pool(name="psagg", bufs=1, space="PSUM"))
            ps_misc = ctx.enter_context(tc.tile_pool(name="psmisc", bufs=1, space="PSUM"))

            # ---- packed constant loads (3 DMAs) ----
            bfp = const.tile([128, nbf], BF16)
            nc.gpsimd.dma_start(bfp[:], bfp_d[:])
            fp = const.tile([128, nf32], F32)
            nc.gpsimd.dma_start(fp[:], fp_d[:])
            idx_sb = const.tile([128, PAIRS // 16], I16)
            nc.gpsimd.dma_start(idx_sb[:], idx_d[:])

            xt_sb = bfp[:, BC_XT:BC_XT + 256]
            w2_sb = bfp[:, BC_W2:BC_W2 + 128]
            win_sb = bfp[:, BC_WIN:BC_WIN + 128]
            wout_sb = bfp[:, BC_WOUT:BC_WOUT + 128]
            w1_sb = bfp[0:G, BC_W1:BC_W1 + 128]
            rt_sb = fp[:, FC_RT:FC_RT + 256]
            b1_ap = fp[:, C_B1:C_B1 + 1]
            bout_ap = fp[:, C_BOUT:C_BOUT + 1]
            pih_ap = fp[:, C_PIH:C_PIH + 1]
            half_ap = fp[:, C_HALF:C_HALF + 1]

            # ---- stage A: y = x @ W_in2f  (atom-major, bf16 DRAM table) ----
            psum_y = ps_misc.tile([128, 2, 128], F32, tag="pmisc")
            for ah in range(2):
                nc.tensor.matmul(
                    psum_y[:, ah, :],
                    xt_sb[:, ah * 128:(ah + 1) * 128],
                    win_sb,
                )
            y_sb = const.tile([128, 2, 128], BF16)
            nc.vector.tensor_copy(y_sb[:], psum_y[:])
            ytab_v = ytab_d[:].rearrange("(h p) f -> h p f", h=2)
            for ah in range(2):
                nc.gpsimd.dma_start(ytab_v[ah], y_sb[:, ah, :])

            # ---- stage A2: cutoff Cm (NBH-part, atom-free) ----
            rcl = mpool.tile([NBH, A], F32)
            nc.vector.tensor_scalar(rcl[:], rt_sb, CUTOFF, None,
                                    op0=mybir.AluOpType.min)
            c1 = mpool.tile([NBH, A], F32)
            nc.scalar.activation(c1[:], rcl[:], mybir.ActivationFunctionType.Sin,
                                 bias=pih_ap, scale=float(-math.pi / CUTOFF))
            cm_f = mpool.tile([NBH, A], F32)
            nc.vector.tensor_scalar(cm_f[:], c1[:], 0.5, 0.5,
                                    op0=mybir.AluOpType.mult,
                                    op1=mybir.AluOpType.add)
            lt = mpool.tile([NBH, A], F32)
            nc.vector.tensor_scalar(lt[:], rt_sb, CUTOFF, None,
                                    op0=mybir.AluOpType.is_lt)
            nc.vector.tensor_tensor(cm_f[:], cm_f[:], lt[:],
                                    op=mybir.AluOpType.mult)
            if use_mask:
                nc.vector.tensor_tensor(cm_f[:], cm_f[:],
                                        fp[:, fc_mask:fc_mask + 256],
                                        op=mybir.AluOpType.mult)
            cmb = const.tile([NBH, A], BF16)
            nc.vector.tensor_copy(cmb[:], cm_f[:])

            # ---- pipelined blocks: mm1+ssp | gather | mm2+V+reduce ----
            hh_sb = hpool.tile([128, PAIRS], BF16)
            yg_sb = ygpool.tile([128, A, F], BF16)
            ps_aggT = ps_agg.tile([128, A], F32)

            fij_cur = fpool.tile([G, 4096], BF16)
            nc.gpsimd.dma_start(fij_cur[:], fijT_d[:, 0:4096])
            for i in range(NBLK):
                # B: mm1 + exact ssp on the prefetched f_ij block
                fij_t = fij_cur
                u_t = upool.tile([128, 4096], BF16)
                for j in range(4):
                    ph = ps_h.tile([128, 1024], F32)
                    for k in range(2):
                        o0 = j * 1024 + k * 512
                        nc.tensor.matmul(
                            ph[:, k * 512:(k + 1) * 512],
                            w1_sb,
                            fij_t[:, o0:o0 + 512],
                        )
                    nc.scalar.activation(u_t[:, j * 1024:(j + 1) * 1024],
                                         ph[:], mybir.ActivationFunctionType.Exp,
                                         bias=b1_ap, scale=1.0)
                nc.scalar.activation(
                    hh_sb[:, i * 4096:(i + 1) * 4096], u_t[:],
                    mybir.ActivationFunctionType.Ln,
                    bias=half_ap, scale=0.5)

                # prefetch next block's f_ij ahead of this block's gathers
                # so the ACT-feeding mm1 chain never queues behind them
                if i + 1 < NBLK:
                    fij_cur = fpool.tile([G, 4096], BF16)
                    nc.gpsimd.dma_start(
                        fij_cur[:], fijT_d[:, (i + 1) * 4096:(i + 2) * 4096])

                # C: gather this block's 4096 neighbor rows, 1024 per
                # instruction (HW limit: 64 descriptors/engine per gather).
                for g in range(4 * i, 4 * i + 4):
                    nc.gpsimd.dma_gather(
                        out_ap=yg_sb[:, g * 8:(g + 1) * 8, :],
                        in_ap=ytab_d[:],
                        idxs_ap=idx_sb[:, g * 64:(g + 1) * 64],
                        num_idxs=1024,
                        num_idxs_reg=1024,
                        elem_size=F,
                    )

                # D: filter-multiply-reduce for this block
                for sc in range(NSC_BLK * i, NSC_BLK * (i + 1)):
                    pm2 = ps_m2.tile([128, 512], F32)
                    for k in range(4):
                        c = 4 * sc + k
                        nc.tensor.matmul(
                            pm2[:, k * 128:(k + 1) * 128],
                            hh_sb[:, c * 128:(c + 1) * 128],
                            w2_sb,
                        )
                    v_t = vpool.tile([128, 4, 128], BF16)
                    nc.vector.tensor_tensor(
                        v_t[:], yg_sb[:, 4 * sc:4 * sc + 4, :],
                        pm2[:].rearrange("p (c f) -> p c f", f=128),
                        op=mybir.AluOpType.mult)
                    for k in range(4):
                        a = 4 * sc + k
                        nc.tensor.matmul(ps_aggT[:, a:a + 1], v_t[:, k, :],
                                         cmb[:, a:a + 1])

            # ---- stage E: b2 correction, out-projection, final ssp ----
            aggf = mpool.tile([128, A], BF16)
            if use_b2:
                prt = ps_misc.tile([128, A], F32, tag="pmisc")
                for jh in range(2):
                    nc.tensor.matmul(prt[:], y_sb[:, jh, :],
                                     bfp[:, BC_TT + jh * 256:BC_TT + (jh + 1) * 256],
                                     start=(jh == 0), stop=(jh == 1))
                rt2_sb = mpool.tile([128, A], F32)
                nc.vector.tensor_copy(rt2_sb[:], prt[:])
                nc.vector.scalar_tensor_tensor(
                    out=aggf[:], in0=rt2_sb[:], scalar=fp[:, C_B2:C_B2 + 1],
                    in1=ps_aggT[:],
                    op0=mybir.AluOpType.mult, op1=mybir.AluOpType.add)
            else:
                nc.vector.tensor_copy(aggf[:], ps_aggT[:])

            po = ps_misc.tile([128, A], F32, tag="pmisc")
            nc.tensor.matmul(po[:], wout_sb, aggf[:])
            u2 = mpool.tile([128, A], F32)
            nc.scalar.activation(u2[:], po[:], mybir.ActivationFunctionType.Exp,
                                 bias=bout_ap, scale=1.0)
            oT = mpool.tile([128, A], F32)
            nc.scalar.activation(oT[:], u2[:], mybir.ActivationFunctionType.Ln,
                                 bias=half_ap, scale=0.5)
            nc.gpsimd.dma_start(outT_d[:], oT[:])

    nc.finalize()
    return nc


def _make_runner(nc):
    """Jit the SPMD executable once; reuse across kernel() calls."""
    import jax
    from jax.sharding import Mesh, PartitionSpec
    from jax.experimental.shard_map import shard_map
    from concourse import bass2jax
    from concourse import mybir as mb

    bass2jax.install_neuronx_cc_hook()

    pid_name = nc.partition_id_tensor.name if nc.partition_id_tensor else None
    in_names, out_names, out_avals, zero_shapes = [], [], [], []
    for alloc in nc.m.functions[0].allocations:
        if not isinstance(alloc, mb.MemoryLocationSet):
            continue
        name = alloc.memorylocations[0].name
        if alloc.kind == "ExternalInput":
            if name != pid_name:
                in_names.append(name)
        elif alloc.kind == "ExternalOutput":
            shape = tuple(alloc.tensor_shape)
            dtype = mb.dt.np(alloc.dtype)
            out_names.append(name)
            out_avals.append(jax.core.ShapedArray(shape, dtype))
            zero_shapes.append((shape, dtype))
    n_params = len(in_names)
    all_in = in_names + out_names
    if pid_name is not None:
        all_in = all_in + [pid_name]
    donate = tuple(range(n_params, n_params + len(out_names)))

    def _body(*args):
        operands = list(args)
        if pid_name is not None:
            operands.append(bass2jax.partition_id_tensor())
        outs = bass2jax._bass_exec_p.bind(
            *operands,
            out_avals=tuple(out_avals),
            in_names=tuple(all_in),
            out_names=tuple(out_names),
            lowering_input_output_aliases=(),
            sim_require_finite=True,
            sim_require_nnan=True,
            nc=nc,
        )
        return tuple(outs)

    devices = jax.devices()[:B]
    mesh = Mesh(np.asarray(devices), ("core",))
    nin = n_params + len(out_names)
    sharded = jax.jit(
        shard_map(_body, mesh=mesh,
                  in_specs=(PartitionSpec("core"),) * nin,
                  out_specs=(PartitionSpec("core"),) * len(out_names),
                  check_rep=False),
        donate_argnums=donate, keep_unused=True)

    def run(in_maps):
        concat_in = [
            np.concatenate([np.asarray(in_maps[c][n]) for c in range(B)], axis=0)
            for n in in_names
        ]
        zeros = [np.zeros((B * s[0], *s[1:]), d) for s, d in zero_shapes]
        out_arrs = sharded(*concat_in, *zeros)
        return [
            {n: np.asarray(out_arrs[i]).reshape(B, *out_avals[i].shape)[c]
             for i, n in enumerate(out_names)}
            for c in range(B)
        ]

    run.sharded = sharded
    run.in_names = in_names
    run.zero_shapes = zero_shapes
    return run


def _prep_shared(W1, W2, W_in2f, W_out, b1, b2, b_out, use_b2, use_mask):
    nbf = 768 + (512 if use_b2 else 0)
    bfp = np.zeros((128, nbf), ml_dtypes.bfloat16)
    bfp[:, BC_W2:BC_W2 + 128] = W2.astype(ml_dtypes.bfloat16)
    bfp[:, BC_WIN:BC_WIN + 128] = W_in2f.astype(ml_dtypes.bfloat16)
    bfp[:, BC_WOUT:BC_WOUT + 128] = W_out.astype(ml_dtypes.bfloat16)
    bfp[0:G, BC_W1:BC_W1 + 128] = W1.astype(ml_dtypes.bfloat16)

    fc_sc = 256 + (256 if use_mask else 0)
    nf32 = fc_sc + 5
    fp = np.zeros((128, nf32), np.float32)
    fp[:, fc_sc + 0] = b1
    fp[:, fc_sc + 1] = b_out
    fp[:, fc_sc + 2] = np.pi / 2
    fp[:, fc_sc + 3] = 0.5
    if use_b2:
        fp[:, fc_sc + 4] = b2
    return bfp, fp, fc_sc


def _prep_core(b, x, r_ij, nbh, mask, f_ij, bfp, fp, use_b2, use_mask):
    """Host-side per-core input marshalling (layout only + index preproc)."""
    m = {}
    m["fijT"] = np.ascontiguousarray(
        f_ij[b].reshape(PAIRS, G).T).astype(ml_dtypes.bfloat16)
    flat = nbh[b].reshape(PAIRS).astype(np.int16)
    idx16 = np.ascontiguousarray(flat.reshape(PAIRS // 16, 16).T)  # (16, 2048)
    m["idx16"] = np.tile(idx16, (8, 1))

    bfp_c = bfp.copy()
    bfp_c[:, BC_XT:BC_XT + 256] = x[b].T.astype(ml_dtypes.bfloat16)
    if use_b2:
        cm = (0.5 * (np.cos(r_ij[b] * (np.pi / CUTOFF)) + 1.0)
              * (r_ij[b] < CUTOFF) * mask[b]).astype(np.float32)  # (A, NBH)
        T = np.zeros((A, A), np.float32)
        np.add.at(T, (np.repeat(np.arange(A), NBH), nbh[b].reshape(-1)),
                  cm.reshape(-1))
        bfp_c[:, BC_TT:BC_TT + 512] = np.concatenate(
            [T.T[0:128], T.T[128:256]], axis=1).astype(ml_dtypes.bfloat16)
    m["bfpack"] = bfp_c

    fp_c = fp.copy()
    fp_c[:, FC_RT:FC_RT + 256] = r_ij[b].T.astype(np.float32)
    if use_mask:
        fp_c[:, 256:512] = mask[b].T.astype(np.float32)
    m["fpack"] = fp_c
    return m


def kernel(**inputs) -> np.ndarray:
    global _last_results
    x = np.asarray(inputs["x"], np.float32)
    r_ij = np.asarray(inputs["r_ij"], np.float32)
    nbh = np.asarray(inputs["neighbors"])
    mask = np.asarray(inputs["pairwise_mask"], np.float32)
    f_ij = np.asarray(inputs["f_ij"], np.float32)
    W1 = np.asarray(inputs["W1"], np.float32)
    b1 = np.asarray(inputs["b1"], np.float32)
    W2 = np.asarray(inputs["W2"], np.float32)
    b2 = np.asarray(inputs["b2"], np.float32)
    W_in2f = np.asarray(inputs["W_in2f"], np.float32)
    W_out = np.asarray(inputs["W_out"], np.float32)
    b_out = np.asarray(inputs["b_out"], np.float32)

    use_b2 = bool(np.any(b2 != 0.0))
    use_mask = bool(np.any(mask != 1.0))

    key = (use_b2, use_mask)
    if key not in _prog_cache:
        _prog_cache[key] = _build(use_b2, use_mask)
    nc = _prog_cache[key]
    if key not in _runner_cache:
        _runner_cache[key] = _make_runner(nc)
    runner = _runner_cache[key]

    bfp, fp, _ = _prep_shared(W1, W2, W_in2f, W_out, b1, b2, b_out,
                              use_b2, use_mask)
    in_maps = [
        _prep_core(b, x, r_ij, nbh, mask, f_ij, bfp, fp, use_b2, use_mask)
        for b in range(B)
    ]

    if os.environ.get("CFCONV_TRACE"):
        res = run_bass_kernel_spmd(nc, in_maps, list(range(B)), trace=True)
        _last_results = res
        results = res.results
    else:
        results = runner(in_maps)
    out = np.stack([np.asarray(results[b]["outT"]).T for b in range(B)])
    return out.astype(np.float32)



# revision 5
# speedup vs baseline: 8.2411x; 8.2411x over previous
"""CFConv (SchNet continuous-filter convolution) Trainium2 Bass kernel.

Problem: nn_CFConv_44332652429581 (gnn_message_passing, 8 cores).

Reference computation (per batch element b):
    W    = ssp(f_ij @ W1 + b1) @ W2 + b2          # filter net, (A,NBH,F)
    C    = 0.5*(cos(pi*r/5)+1)*(r<5)              # cosine cutoff, (A,NBH)
    Wc   = W * C * mask
    y    = x @ W_in2f                              # (A,F)
    agg  = sum_n  y[nbh[a,n]] * Wc[a,n]            # (A,F)
    out  = ssp(agg @ W_out + b_out)                # (A,O)
where ssp(v) = softplus(v) - ln 2.

Sharding: data-parallel over the batch axis, one batch element per core
(B=8 == n_cores). No collectives.

Per-core dataflow (pairs = A*NBH = 32768; "chunk" = one atom's 128
neighbors; "block" = 4096 pairs = 32 atoms). Blocks are pipelined:
load f_ij block / mm1 / ssp, gather that block's neighbor rows, then
filter-multiply-reduce it, all stages overlapping across blocks.

  - mm1 F-major: psum_h = W1^T @ f_ij^T            (bf16, W1 stationary)
  - ssp exactly in two ACT passes: u = Exp(h1+b1); h' = Ln(0.5u+0.5)
    (Ln(0.5e^v+0.5) == softplus(v) - ln2 exactly, so W = h'@W2 + b2)
  - y table (A,F) bf16 in DRAM; neighbor rows fetched with dma_gather,
    1024 rows per instruction (SWDGE ring holds 128 descs/engine)
    -> yg pairs-major, chunk == atom
  - mm2 per chunk: lhsT = h' slice (weights), rhs = W2 -> M2 (pairs,F) PSUM
  - V = yg * M2 on DVE (one pass per 512-pair superchunk)
  - neighbor reduce as one matmul per atom: aggT[:, a] = V_a^T @ Cm_a
    (the cutoff+mask vector Cm rides along as the reduce weights)
  - b2 correction (only when b2 != 0): agg += b2 (x)_f R, R^T = y^T @ T^T
    with T[a,j] = sum_{n: nbh[a,n]=j} Cm[a,n] host-precomputed from
    (neighbors, r_ij, mask) -- pure input preprocessing.
  - out-proj: psum_o = W_out^T @ aggT, final ssp via Exp/Ln,
    store outT (O, A); host transposes per core on unshard.

All DMA goes through gpsimd (SWDGE): the HWDGE rings (sync/scalar
engines) do not function on the axon PJRT runtime this kernel targets.
"""
import math
import os

import numpy as np
import ml_dtypes

import concourse.bass as bass
import concourse.tile as tile
from concourse import bacc, mybir
from concourse.bass_utils import run_bass_kernel_spmd


def _patch_act_tables():
    """Prefer the combined Exp+Ln activation table so the ACT engine does
    not thrash 1.3us table reloads between the softplus Exp and Ln passes."""
    if getattr(bacc, "_cfconv_act_patch", False):
        return
    orig = bacc.get_activation_tables

    def patched(arch):
        # Table ids are positional: keep the dict order identical, but strip
        # Exp/Ln from the single-transcendental tables so the chooser must
        # pick the combined natural_log_exp table for both passes.
        t = dict(orig(arch))
        exp_t = mybir.ActivationFunctionType.Exp
        ln_t = mybir.ActivationFunctionType.Ln
        out = {}
        for k, funcs in t.items():
            if k != "natural_log_exp_and_others" and (
                    exp_t in funcs or ln_t in funcs):
                funcs = funcs - {exp_t, ln_t}
            out[k] = funcs
        return out

    bacc.get_activation_tables = patched
    bacc._cfconv_act_patch = True

F32 = mybir.dt.float32
BF16 = mybir.dt.bfloat16
I16 = mybir.dt.int16

B, A, NBH = 8, 256, 128
G, F, O = 25, 128, 128
PAIRS = A * NBH            # 32768
CUTOFF = 5.0
NBLK = 8                   # pipeline blocks of 4096 pairs
NSC_BLK = 8                # superchunks (512 pairs) per block

# bf16 const-pack column layout
BC_XT = 0          # xT (128, 256)
BC_W2 = 256        # W2 (128, 128)
BC_WIN = 384       # W_in2f (128, 128)
BC_WOUT = 512      # W_out (128, 128)
BC_W1 = 640        # W1 padded to (128, 128); rows 0:25 valid
BC_TT = 768        # TT (2x (128, 256)) when use_b2

FC_RT = 0          # rT (128, 256) in the f32 pack

_prog_cache = {}
_runner_cache = {}
_last_results = None       # test.py introspection


def _build(use_b2: bool, use_mask: bool, n_repeat: int = 1):
    """Build + compile the per-core Bass program (SPMD; same for all cores).

    n_repeat > 1 repeats the ENTIRE body (constant loads included) that many
    times inside one NEFF.  Used only for timing: the per-execution device
    time is the slope of wall time vs n_repeat, which cancels the ~1.2 ms
    per-dispatch overhead of the axon PJRT tunnel.
    """
    from contextlib import ExitStack

    nbf = 768 + (512 if use_b2 else 0)
    fc_mask = 256
    fc_sc = 256 + (256 if use_mask else 0)   # scalar column block start
    nf32 = fc_sc + 5
    C_B1, C_BOUT, C_PIH, C_HALF, C_B2 = range(fc_sc, fc_sc + 5)

    _patch_act_tables()
    nc = bacc.Bacc("TRN2")

    fijT_d = nc.dram_tensor("fijT", [G, PAIRS], BF16, kind="ExternalInput")
    idx_d = nc.dram_tensor("idx16", [128, PAIRS // 16], I16, kind="ExternalInput")
    bfp_d = nc.dram_tensor("bfpack", [128, nbf], BF16, kind="ExternalInput")
    fp_d = nc.dram_tensor("fpack", [128, nf32], F32, kind="ExternalInput")

    outT_d = nc.dram_tensor("outT", [O, A], F32, kind="ExternalOutput")
    ytab_d = nc.dram_tensor("ytab", [A, F], BF16)  # internal

    with tile.TileContext(nc) as tc:
        with ExitStack() as ctx:
            const = ctx.enter_context(tc.tile_pool(name="const", bufs=1))
            fpool = ctx.enter_context(tc.tile_pool(name="fij", bufs=4))
            upool = ctx.enter_context(tc.tile_pool(name="u", bufs=3))
            hpool = ctx.enter_context(tc.tile_pool(name="hh", bufs=1))
            ygpool = ctx.enter_context(tc.tile_pool(name="yg", bufs=1))
            vpool = ctx.enter_context(tc.tile_pool(name="v", bufs=6))
            mpool = ctx.enter_context(tc.tile_pool(name="misc", bufs=1))
            ps_h = ctx.enter_context(tc.tile_pool(name="psh", bufs=2, space="PSUM"))
            ps_m2 = ctx.enter_context(tc.tile_pool(name="psm2", bufs=2, space="PSUM"))
            ps_agg = ctx.enter_context(tc.tile_pool(name="psagg", bufs=1, space="PSUM"))
            ps_misc = ctx.enter_context(tc.tile_pool(name="psmisc", bufs=1, space="PSUM"))

            for _rep in range(n_repeat):
                _build_body(nc, tc, use_b2, use_mask,
                            fijT_d, idx_d, bfp_d, fp_d, outT_d, ytab_d,
                            const, fpool, upool, hpool, ygpool, vpool, mpool,
                            ps_h, ps_m2, ps_agg, ps_misc,
                            nbf, fc_mask, fc_sc,
                            C_B1, C_BOUT, C_PIH, C_HALF, C_B2)

    nc.finalize()
    return nc


def _build_body(nc, tc, use_b2, use_mask,
                fijT_d, idx_d, bfp_d, fp_d, outT_d, ytab_d,
                const, fpool, upool, hpool, ygpool, vpool, mpool,
                ps_h, ps_m2, ps_agg, ps_misc,
                nbf, fc_mask, fc_sc,
                C_B1, C_BOUT, C_PIH, C_HALF, C_B2):
    nf32 = fc_sc + 5
    if True:
        if True:
            # ---- packed constant loads (3 DMAs) ----
            bfp = const.tile([128, nbf], BF16)
            nc.gpsimd.dma_start(bfp[:], bfp_d[:])
            fp = const.tile([128, nf32], F32)
            nc.gpsimd.dma_start(fp[:], fp_d[:])
            idx_sb = const.tile([128, PAIRS // 16], I16)
            nc.gpsimd.dma_start(idx_sb[:], idx_d[:])

            xt_sb = bfp[:, BC_XT:BC_XT + 256]
            w2_sb = bfp[:, BC_W2:BC_W2 + 128]
            win_sb = bfp[:, BC_WIN:BC_WIN + 128]
            wout_sb = bfp[:, BC_WOUT:BC_WOUT + 128]
            w1_sb = bfp[0:G, BC_W1:BC_W1 + 128]
            rt_sb = fp[:, FC_RT:FC_RT + 256]
            b1_ap = fp[:, C_B1:C_B1 + 1]
            bout_ap = fp[:, C_BOUT:C_BOUT + 1]
            pih_ap = fp[:, C_PIH:C_PIH + 1]
            half_ap = fp[:, C_HALF:C_HALF + 1]

            # ---- stage A: y = x @ W_in2f  (atom-major, bf16 DRAM table) ----
            psum_y = ps_misc.tile([128, 2, 128], F32, tag="pmisc")
            for ah in range(2):
                nc.tensor.matmul(
                    psum_y[:, ah, :],
                    xt_sb[:, ah * 128:(ah + 1) * 128],
                    win_sb,
                )
            y_sb = const.tile([128, 2, 128], BF16)
            nc.vector.tensor_copy(y_sb[:], psum_y[:])
            ytab_v = ytab_d[:].rearrange("(h p) f -> h p f", h=2)
            for ah in range(2):
                nc.gpsimd.dma_start(ytab_v[ah], y_sb[:, ah, :])

            # ---- stage A2: cutoff Cm (NBH-part, atom-free) ----
            rcl = mpool.tile([NBH, A], F32)
            nc.vector.tensor_scalar(rcl[:], rt_sb, CUTOFF, None,
                                    op0=mybir.AluOpType.min)
            c1 = mpool.tile([NBH, A], F32)
            nc.scalar.activation(c1[:], rcl[:], mybir.ActivationFunctionType.Sin,
                                 bias=pih_ap, scale=float(-math.pi / CUTOFF))
            cm_f = mpool.tile([NBH, A], F32)
            nc.vector.tensor_scalar(cm_f[:], c1[:], 0.5, 0.5,
                                    op0=mybir.AluOpType.mult,
                                    op1=mybir.AluOpType.add)
            lt = mpool.tile([NBH, A], F32)
            nc.vector.tensor_scalar(lt[:], rt_sb, CUTOFF, None,
                                    op0=mybir.AluOpType.is_lt)
            nc.vector.tensor_tensor(cm_f[:], cm_f[:], lt[:],
                                    op=mybir.AluOpType.mult)
            if use_mask:
                nc.vector.tensor_tensor(cm_f[:], cm_f[:],
                                        fp[:, fc_mask:fc_mask + 256],
                                        op=mybir.AluOpType.mult)
            cmb = const.tile([NBH, A], BF16)
            nc.vector.tensor_copy(cmb[:], cm_f[:])

            # ---- pipelined blocks: mm1+ssp | gather | mm2+V+reduce ----
            hh_sb = hpool.tile([128, PAIRS], BF16)
            yg_sb = ygpool.tile([128, A, F], BF16)
            ps_aggT = ps_agg.tile([128, A], F32)

            fij_cur = fpool.tile([G, 4096], BF16)
            nc.gpsimd.dma_start(fij_cur[:], fijT_d[:, 0:4096])
            for i in range(NBLK):
                # B: mm1 + exact ssp on the prefetched f_ij block
                fij_t = fij_cur
                u_t = upool.tile([128, 4096], BF16)
                for j in range(4):
                    ph = ps_h.tile([128, 1024], F32)
                    for k in range(2):
                        o0 = j * 1024 + k * 512
                        nc.tensor.matmul(
                            ph[:, k * 512:(k + 1) * 512],
                            w1_sb,
                            fij_t[:, o0:o0 + 512],
                        )
                    nc.scalar.activation(u_t[:, j * 1024:(j + 1) * 1024],
                                         ph[:], mybir.ActivationFunctionType.Exp,
                                         bias=b1_ap, scale=1.0)
                nc.scalar.activation(
                    hh_sb[:, i * 4096:(i + 1) * 4096], u_t[:],
                    mybir.ActivationFunctionType.Ln,
                    bias=half_ap, scale=0.5)

                # prefetch next block's f_ij ahead of this block's gathers
                # so the ACT-feeding mm1 chain never queues behind them
                if i + 1 < NBLK:
                    fij_cur = fpool.tile([G, 4096], BF16)
                    nc.gpsimd.dma_start(
                        fij_cur[:], fijT_d[:, (i + 1) * 4096:(i + 2) * 4096])

                # C: gather this block's 4096 neighbor rows, 1024 per
                # instruction (HW limit: 64 descriptors/engine per gather).
                for g in range(4 * i, 4 * i + 4):
                    nc.gpsimd.dma_gather(
                        out_ap=yg_sb[:, g * 8:(g + 1) * 8, :],
                        in_ap=ytab_d[:],
                        idxs_ap=idx_sb[:, g * 64:(g + 1) * 64],
                        num_idxs=1024,
                        num_idxs_reg=1024,
                        elem_size=F,
                    )

                # D: filter-multiply-reduce for this block
                for sc in range(NSC_BLK * i, NSC_BLK * (i + 1)):
                    pm2 = ps_m2.tile([128, 512], F32)
                    for k in range(4):
                        c = 4 * sc + k
                        nc.tensor.matmul(
                            pm2[:, k * 128:(k + 1) * 128],
                            hh_sb[:, c * 128:(c + 1) * 128],
                            w2_sb,
                        )
                    v_t = vpool.tile([128, 4, 128], BF16)
                    nc.vector.tensor_tensor(
                        v_t[:], yg_sb[:, 4 * sc:4 * sc + 4, :],
                        pm2[:].rearrange("p (c f) -> p c f", f=128),
                        op=mybir.AluOpType.mult)
                    for k in range(4):
                        a = 4 * sc + k
                        nc.tensor.matmul(ps_aggT[:, a:a + 1], v_t[:, k, :],
                                         cmb[:, a:a + 1])

            # ---- stage E: b2 correction, out-projection, final ssp ----
            aggf = mpool.tile([128, A], BF16)
            if use_b2:
                prt = ps_misc.tile([128, A], F32, tag="pmisc")
                for jh in range(2):
                    nc.tensor.matmul(prt[:], y_sb[:, jh, :],
                                     bfp[:, BC_TT + jh * 256:BC_TT + (jh + 1) * 256],
                                     start=(jh == 0), stop=(jh == 1))
                rt2_sb = mpool.tile([128, A], F32)
                nc.vector.tensor_copy(rt2_sb[:], prt[:])
                nc.vector.scalar_tensor_tensor(
                    out=aggf[:], in0=rt2_sb[:], scalar=fp[:, C_B2:C_B2 + 1],
                    in1=ps_aggT[:],
                    op0=mybir.AluOpType.mult, op1=mybir.AluOpType.add)
            else:
                nc.vector.tensor_copy(aggf[:], ps_aggT[:])

            po = ps_misc.tile([128, A], F32, tag="pmisc")
            nc.tensor.matmul(po[:], wout_sb, aggf[:])
            u2 = mpool.tile([128, A], F32)
            nc.scalar.activation(u2[:], po[:], mybir.ActivationFunctionType.Exp,
                                 bias=bout_ap, scale=1.0)
            oT = mpool.tile([128, A], F32)
            nc.scalar.activation(oT[:], u2[:], mybir.ActivationFunctionType.Ln,
                                 bias=half_ap, scale=0.5)
            nc.gpsimd.dma_start(outT_d[:], oT[:])


def _make_runner(nc):
    """Jit the SPMD executable once; reuse across kernel() calls."""
    import jax
    from jax.sharding import Mesh, PartitionSpec
    from jax.experimental.shard_map import shard_map
    from concourse import bass2jax
    from concourse import mybir as mb

    bass2jax.install_neuronx_cc_hook()

    pid_name = nc.partition_id_tensor.name if nc.partition_id_tensor else None
    in_names, out_names, out_avals, zero_shapes = [], [], [], []
    for alloc in nc.m.functions[0].allocations:
        if not isinstance(alloc, mb.MemoryLocationSet):
            continue
        name = alloc.memorylocations[0].name
        if alloc.kind == "ExternalInput":
            if name != pid_name:
                in_names.append(name)
        elif alloc.kind == "ExternalOutput":
            shape = tuple(alloc.tensor_shape)
            dtype = mb.dt.np(alloc.dtype)
            out_names.append(name)
            out_avals.append(jax.core.ShapedArray(shape, dtype))
            zero_shapes.append((shape, dtype))
    n_params = len(in_names)
    all_in = in_names + out_names
    if pid_name is not None:
        all_in = all_in + [pid_name]
    donate = tuple(range(n_params, n_params + len(out_names)))

    def _body(*args):
        operands = list(args)
        if pid_name is not None:
            operands.append(bass2jax.partition_id_tensor())
        outs = bass2jax._bass_exec_p.bind(
            *operands,
            out_avals=tuple(out_avals),
            in_names=tuple(all_in),
            out_names=tuple(out_names),
            lowering_input_output_aliases=(),
            sim_require_finite=True,
            sim_require_nnan=True,
            nc=nc,
        )
        return tuple(outs)

    devices = jax.devices()[:B]
    mesh = Mesh(np.asarray(devices), ("core",))
    nin = n_params + len(out_names)
    sharded = jax.jit(
        shard_map(_body, mesh=mesh,
                  in_specs=(PartitionSpec("core"),) * nin,
                  out_specs=(PartitionSpec("core"),) * len(out_names),
                  check_rep=False),
        donate_argnums=donate, keep_unused=True)

    def run(in_maps):
        concat_in = [
            np.concatenate([np.asarray(in_maps[c][n]) for c in range(B)], axis=0)
            for n in in_names
        ]
        zeros = [np.zeros((B * s[0], *s[1:]), d) for s, d in zero_shapes]
        out_arrs = sharded(*concat_in, *zeros)
        return [
            {n: np.asarray(out_arrs[i]).reshape(B, *out_avals[i].shape)[c]
             for i, n in enumerate(out_names)}
            for c in range(B)
        ]

    run.sharded = sharded
    run.in_names = in_names
    run.zero_shapes = zero_shapes
    return run


def _prep_shared(W1, W2, W_in2f, W_out, b1, b2, b_out, use_b2, use_mask):
    nbf = 768 + (512 if use_b2 else 0)
    bfp = np.zeros((128, nbf), ml_dtypes.bfloat16)
    bfp[:, BC_W2:BC_W2 + 128] = W2.astype(ml_dtypes.bfloat16)
    bfp[:, BC_WIN:BC_WIN + 128] = W_in2f.astype(ml_dtypes.bfloat16)
    bfp[:, BC_WOUT:BC_WOUT + 128] = W_out.astype(ml_dtypes.bfloat16)
    bfp[0:G, BC_W1:BC_W1 + 128] = W1.astype(ml_dtypes.bfloat16)

    fc_sc = 256 + (256 if use_mask else 0)
    nf32 = fc_sc + 5
    fp = np.zeros((128, nf32), np.float32)
    fp[:, fc_sc + 0] = b1
    fp[:, fc_sc + 1] = b_out
    fp[:, fc_sc + 2] = np.pi / 2
    fp[:, fc_sc + 3] = 0.5
    if use_b2:
        fp[:, fc_sc + 4] = b2
    return bfp, fp, fc_sc


def _prep_core(b, x, r_ij, nbh, mask, f_ij, bfp, fp, use_b2, use_mask):
    """Host-side per-core input marshalling (layout only + index preproc)."""
    m = {}
    m["fijT"] = np.ascontiguousarray(
        f_ij[b].reshape(PAIRS, G).T).astype(ml_dtypes.bfloat16)
    flat = nbh[b].reshape(PAIRS).astype(np.int16)
    idx16 = np.ascontiguousarray(flat.reshape(PAIRS // 16, 16).T)  # (16, 2048)
    m["idx16"] = np.tile(idx16, (8, 1))

    bfp_c = bfp.copy()
    bfp_c[:, BC_XT:BC_XT + 256] = x[b].T.astype(ml_dtypes.bfloat16)
    if use_b2:
        cm = (0.5 * (np.cos(r_ij[b] * (np.pi / CUTOFF)) + 1.0)
              * (r_ij[b] < CUTOFF) * mask[b]).astype(np.float32)  # (A, NBH)
        T = np.zeros((A, A), np.float32)
        np.add.at(T, (np.repeat(np.arange(A), NBH), nbh[b].reshape(-1)),
                  cm.reshape(-1))
        bfp_c[:, BC_TT:BC_TT + 512] = np.concatenate(
            [T.T[0:128], T.T[128:256]], axis=1).astype(ml_dtypes.bfloat16)
    m["bfpack"] = bfp_c

    fp_c = fp.copy()
    fp_c[:, FC_RT:FC_RT + 256] = r_ij[b].T.astype(np.float32)
    if use_mask:
        fp_c[:, 256:512] = mask[b].T.astype(np.float32)
    m["fpack"] = fp_c
    return m


def kernel(**inputs) -> np.ndarray:
    global _last_results
    x = np.asarray(inputs["x"], np.float32)
    r_ij = np.asarray(inputs["r_ij"], np.float32)
    nbh = np.asarray(inputs["neighbors"])
    mask = np.asarray(inputs["pairwise_mask"], np.float32)
    f_ij = np.asarray(inputs["f_ij"], np.float32)
    W1 = np.asarray(inputs["W1"], np.float32)
    b1 = np.asarray(inputs["b1"], np.float32)
    W2 = np.asarray(inputs["W2"], np.float32)
    b2 = np.asarray(inputs["b2"], np.float32)
    W_in2f = np.asarray(inputs["W_in2f"], np.float32)
    W_out = np.asarray(inputs["W_out"], np.float32)
    b_out = np.asarray(inputs["b_out"], np.float32)

    use_b2 = bool(np.any(b2 != 0.0))
    use_mask = bool(np.any(mask != 1.0))

    key = (use_b2, use_mask)
    if key not in _prog_cache:
        _prog_cache[key] = _build(use_b2, use_mask)
    nc = _prog_cache[key]
    if key not in _runner_cache:
        _runner_cache[key] = _make_runner(nc)
    runner = _runner_cache[key]

    bfp, fp, _ = _prep_shared(W1, W2, W_in2f, W_out, b1, b2, b_out,
                              use_b2, use_mask)
    in_maps = [
        _prep_core(b, x, r_ij, nbh, mask, f_ij, bfp, fp, use_b2, use_mask)
        for b in range(B)
    ]

    if os.environ.get("CFCONV_TRACE"):
        res = run_bass_kernel_spmd(nc, in_maps, list(range(B)), trace=True)
        _last_results = res
        results = res.results
    else:
        results = runner(in_maps)
    out = np.stack([np.asarray(results[b]["outT"]).T for b in range(B)])
    return out.astype(np.float32)

